# revision 11
# baseline (speedup 1.0000x reference)
"""GATv2 attention layer (B=2, T=1024, C_IN=128, D=64) on 8 trn2 NeuronCores.

Sharding: flatten (B, T) destination rows -> 2048 rows, 256 per core.
q = feat@W1.T and k = feat@W2.T are computed on host (tiny O(T*C) work); the
device does all O(T^2) work. Per-core fp16 host layouts: qT2 = [q^T; q^T]
stacked [128(=2x64 d), 1024(=j)], kpair columns [k(2p); k(2p+1)], the score
weight A32s, feat in 128-row blocks with a ones column (final matmul rhs +
free row-sum), and adjm = (adj-1)*SHIFT (additive softmax mask).

Per-core algorithm (i = destination row, j = source node, d = head dim 64):
  scores[i, j] = sum_d a[d] * relu(q[j, d] + k[i, d])
For a PAIR of rows (2p, 2p+1), bias column kpair[:, p] = [k[2p]; k[2p+1]]:
  E2 = relu(qT2 + kpair[:, p])   one elementwise op, round-robined across
  VectorE (DVE tensor_scalar, 4x fp16 mode), ScalarE (Relu activation) and
  GpSimd (Q7 tensor_scalar) so all three produce E2 tiles concurrently.
Scores come from PE matmuls with lhsT = A32s slot q, a [128, 32] fp16 matrix
holding `a` in columns 2q (top d-half) and 2q+1 (bottom); 16 pairs accumulate
into one 32-row psum band; consecutive pairs hit different PSUM col-groups so
the matmuls overlap on the PE sub-arrays.
The adjacency mask is folded into the psum accumulation: each i-tile starts
with psum := I^T @ adjm (start=True), so masked scores sit at s - SHIFT and
softmax needs no separate multiply:
  att_unnorm = exp(s - BSUB)    (BSUB keeps exp in fp16 range; softmax is
  shift-invariant so the result is exact in fp32 terms)
attT via xbar DMA transpose (blocked 128x128), final: out[i, :] =
(attT.T @ [feat | 1]) with the ones column giving the row-sum, then scale by
its reciprocal.
"""
import sys

sys.path.insert(0, "/opt/trn_rl_repo")

from contextlib import ExitStack

import numpy as np

import concourse.bass as bass  # noqa: F401
import concourse.tile as tile
from concourse import bacc, masks, mybir
from concourse.bass_utils import run_bass_kernel_spmd

B, T, C_IN, D = 2, 1024, 128, 64
N_CORES = 8
ROWS = (B * T) // N_CORES  # 256 destination rows per core
CPB = N_CORES // B  # cores per batch
NT = T // 128  # token tiles
NIT = ROWS // 128  # i-tiles per core
NPAIR = 64  # row pairs per i-tile
NSLOT = 16  # pair slots per 32-row psum band

FP32 = mybir.dt.float32
FP16 = mybir.dt.float16
AX = mybir.AxisListType.X
OP = mybir.AluOpType
AF = mybir.ActivationFunctionType

SHIFT = 40.0  # additive mask magnitude: masked scores -> s - SHIFT
BSUB = 4.0  # global exp shift: att_unnorm = exp(s - BSUB), keeps fp16 range

# Per-i-tile engine assignment for the 64 E2 tiles: greedy balance by
# measured per-tile cost (DVE 396 ns, ScalarE 1150 ns, GpSimd ~1050 ns) with
# initial queue offsets (ScalarE pays the ACT table load / exp of the
# previous i-tile; GpSimd pays adjm DMA issue).
def _make_pattern(wv=396, ws=1150, wg=1050, pv=500, ps=3000, pg=1100):
    cost = {"V": wv, "S": ws, "G": wg}
    load = {"V": pv, "S": ps, "G": pg}
    pat = []
    for _ in range(NPAIR):
        e = min("VSG", key=lambda x: load[x] + cost[x])
        load[e] += cost[e]
        pat.append(e)
    return "".join(pat)


PATTERN = _make_pattern()
PAIRS_PER_ALLOC = 4


def _emit(ctx, tc, nc, qT2_in, kpair_in, a32_in, feat16b_in, adjm_in, out):
    singles = ctx.enter_context(tc.tile_pool(name="singles", bufs=1))
    ident16 = singles.tile([128, 128], FP16)
    qT2 = singles.tile([128, T], FP16)
    kpair = singles.tile([128, ROWS // 2], FP32)
    A32s = singles.tile([128, NSLOT * 32], FP16)
    feat16 = singles.tile([128, NT * (C_IN + 1)], FP16)
    bsub = singles.tile([128, 1], FP32)
    nc.vector.memset(bsub[:], -BSUB)

    # prologue: parallel DMAs on the three DMA-capable queues (sync, scalar,
    # gpsimd); critical path is qT2+kpair
    nc.sync.dma_start(qT2[:], qT2_in[:, :])
    nc.sync.dma_start(kpair[:], kpair_in[:, :])
    nc.scalar.dma_start(A32s[:], a32_in[:, :])
    nc.scalar.dma_start(feat16[:], feat16b_in[:, :])
    masks.make_identity(nc, ident16[:])

    adjpool = ctx.enter_context(tc.tile_pool(name="adjp", bufs=2))
    e2pool = ctx.enter_context(tc.tile_pool(name="e2", bufs=4))
    pattpool = ctx.enter_context(tc.tile_pool(name="patt", bufs=2))
    attTpool = ctx.enter_context(tc.tile_pool(name="attT", bufs=2))
    outpool = ctx.enter_context(tc.tile_pool(name="outp", bufs=2))
    smallpool = ctx.enter_context(tc.tile_pool(name="small", bufs=2))
    ps_scores = ctx.enter_context(tc.tile_pool(name="ps_s", bufs=4, space="PSUM"))
    ps_out = ctx.enter_context(tc.tile_pool(name="ps_o", bufs=2, space="PSUM"))
    ps_tr = ctx.enter_context(tc.tile_pool(name="ps_tr", bufs=1, space="PSUM"))

    st = {}

    def load_adjm(it, eng):
        t = adjpool.tile([128, T], FP16, tag="adjm")
        eng.dma_start(t[:], adjm_in[it * 128 : (it + 1) * 128, :])
        st["adjm", it] = t

    def mask_mm(it):
        # psum := mask (0 / -SHIFT); scores accumulate on top
        s0 = ps_scores.tile([128, 512], FP32, tag="s")
        s1 = ps_scores.tile([128, 512], FP32, tag="s")
        adj_sb = st["adjm", it]
        nc.tensor.matmul(
            s0[:], ident16[:], adj_sb[:, 0:512], start=True, stop=False,
            skip_group_check=True,
        )
        nc.tensor.matmul(
            s1[:], ident16[:], adj_sb[:, 512:T], start=True, stop=False,
            skip_group_check=True,
        )
        st["s", it] = (s0, s1)

    def pairs(it, lo, hi):
        s0, s1 = st["s", it]
        for idx in range(lo, hi):
            q, g = divmod(idx, 4)
            p = NSLOT * g + q
            P = it * NPAIR + p
            sub = idx % PAIRS_PER_ALLOC
            if sub == 0:
                st["e2big"] = e2pool.tile(
                    [128, PAIRS_PER_ALLOC * T], FP16, tag="e2", name="e2big"
                )
            e2 = st["e2big"][:, sub * T : (sub + 1) * T]
            kcol = kpair[:, P : P + 1]
            eng = PATTERN[idx]
            if eng == "S":
                nc.scalar.activation(e2[:], qT2[:], AF.Relu, bias=kcol)
            elif eng == "G":
                nc.gpsimd.tensor_scalar(e2[:], qT2[:], kcol, 0.0, OP.add, OP.max)
            else:
                nc.vector.tensor_scalar(e2[:], qT2[:], kcol, 0.0, OP.add, OP.max)
            lhsT = A32s[:, 32 * q : 32 * q + 32]
            last = q == NSLOT - 1
            nc.tensor.matmul(
                s0[32 * g : 32 * g + 32, :], lhsT, e2[:, 0:512],
                start=False, stop=last,
                tile_position=(0, 32 * g), skip_group_check=True,
            )
            nc.tensor.matmul(
                s1[32 * g : 32 * g + 32, :], lhsT, e2[:, 512:T],
                start=False, stop=last,
                tile_position=(0, 32 * g), skip_group_check=True,
            )

    def tail_exp(it):
        s0, s1 = st["s", it]
        patt = pattpool.tile([128, T], FP16, tag="patt")
        nc.scalar.activation(patt[:, 0:512], s0[:], AF.Exp, bias=bsub[:])
        nc.scalar.activation(patt[:, 512:T], s1[:], AF.Exp, bias=bsub[:])
        st["patt", it] = patt

    def tail_out(it):
        patt = st["patt", it]
        attT = attTpool.tile([128, T], FP16, tag="attT")
        # per-block 128x128 transposes via the DMA xbar on the (idle) sync
        # queue: keeps both PE and DVE free for the concurrent pair loop
        for t in range(NT):
            nc.sync.dma_start_transpose(
                attT[:, t * 128 : (t + 1) * 128], patt[:, t * 128 : (t + 1) * 128]
            )
        W = C_IN + 1
        po = ps_out.tile([128, W], FP32, tag="o")
        for t in range(NT):
            nc.tensor.matmul(
                po[:], attT[:, t * 128 : (t + 1) * 128],
                feat16[:, t * W : (t + 1) * W],
                start=(t == 0), stop=(t == NT - 1),
            )
        inv = smallpool.tile([128, 1], FP32, tag="inv")
        nc.vector.reciprocal(inv[:], po[:, C_IN : C_IN + 1])
        out_sb = outpool.tile([128, C_IN], FP32, tag="out")
        nc.vector.tensor_scalar(out_sb[:], po[:, 0:C_IN], inv[:], None, OP.mult)
        nc.sync.dma_start(out[it * 128 : (it + 1) * 128, :], out_sb[:])

    def tail_last(it):
        # exposed end-of-kernel tail: chunk finely and use PE transposes +
        # DVE copies (all engines are otherwise idle here; latency wins)
        s0, s1 = st["s", it]
        patt = pattpool.tile([128, T], FP16, tag="patt")
        pst = ps_tr.tile([128, T], FP16, tag="tr")
        attT = attTpool.tile([128, T], FP16, tag="attT")
        W = C_IN + 1
        po = ps_out.tile([128, W], FP32, tag="o")
        for hh in range(4):
            lo = hh * 256
            src = (s0, s1)[hh // 2]
            slo = lo % 512
            nc.scalar.activation(
                patt[:, lo : lo + 256], src[:, slo : slo + 256], AF.Exp,
                bias=bsub[:],
            )
            for t in range(lo // 128, lo // 128 + 2):
                nc.tensor.transpose(
                    pst[:, t * 128 : (t + 1) * 128],
                    patt[:, t * 128 : (t + 1) * 128], ident16[:],
                )
            nc.vector.tensor_copy(attT[:, lo : lo + 256], pst[:, lo : lo + 256])
            for t in range(lo // 128, lo // 128 + 2):
                nc.tensor.matmul(
                    po[:], attT[:, t * 128 : (t + 1) * 128],
                    feat16[:, t * W : (t + 1) * W],
                    start=(t == 0), stop=(t == NT - 1),
                )
        inv = smallpool.tile([128, 1], FP32, tag="inv")
        nc.vector.reciprocal(inv[:], po[:, C_IN : C_IN + 1])
        out_sb = outpool.tile([128, C_IN], FP32, tag="out")
        nc.vector.tensor_scalar(out_sb[:], po[:, 0:C_IN], inv[:], None, OP.mult)
        nc.sync.dma_start(out[it * 128 : (it + 1) * 128, :], out_sb[:])

    load_adjm(0, nc.sync)
    mask_mm(0)
    load_adjm(1, nc.gpsimd)
    pairs(0, 0, NPAIR)
    mask_mm(1)
    pairs(1, 0, 16)
    tail_exp(0)
    pairs(1, 16, 32)
    tail_out(0)
    pairs(1, 32, NPAIR)
    tail_last(1)


_PROGRAM = None


def build_program():
    global _PROGRAM
    if _PROGRAM is not None:
        return _PROGRAM
    nc = bacc.Bacc("TRN2", target_bir_lowering=False, debug=False, num_devices=N_CORES)
    qT2_in = nc.dram_tensor("qT2", [128, T], FP16, kind="ExternalInput")
    kpair_in = nc.dram_tensor("kpair", [128, ROWS // 2], FP32, kind="ExternalInput")
    a32_in = nc.dram_tensor("a32", [128, NSLOT * 32], FP16, kind="ExternalInput")
    feat16b_in = nc.dram_tensor(
        "feat16b", [128, NT * (C_IN + 1)], FP16, kind="ExternalInput"
    )
    adjm_in = nc.dram_tensor("adjm", [ROWS, T], FP16, kind="ExternalInput")
    out = nc.dram_tensor("out", [ROWS, C_IN], FP32, kind="ExternalOutput")
    with tile.TileContext(nc) as tc:
        with ExitStack() as ctx:
            _emit(ctx, tc, nc, qT2_in, kpair_in, a32_in, feat16b_in, adjm_in, out)
    nc.compile()
    _PROGRAM = nc
    return nc


def make_a32(a):
    a32 = np.zeros((128, NSLOT * 32), dtype=np.float16)
    for q in range(NSLOT):
        a32[0:64, 32 * q + 2 * q] = a
        a32[64:128, 32 * q + 2 * q + 1] = a
    return a32


def make_in_maps(feat, adj, W1, W2, a):
    feat = np.ascontiguousarray(feat, dtype=np.float32)
    adj = np.ascontiguousarray(adj, dtype=np.float32)
    W1 = np.asarray(W1, dtype=np.float32)
    W2 = np.asarray(W2, dtype=np.float32)
    a32 = make_a32(np.asarray(a, dtype=np.float32))
    in_maps = []
    for b in range(B):
        qb = feat[b] @ W1.T  # [T, D]
        kb = feat[b] @ W2.T  # [T, D]
        qT = np.ascontiguousarray(qb.T.astype(np.float16))  # [D, T]
        qT2 = np.ascontiguousarray(np.concatenate([qT, qT], axis=0))  # [128, T]
        feat16 = feat[b].astype(np.float16)
        fb = feat16.reshape(NT, 128, C_IN).transpose(1, 0, 2)  # [128, NT, C_IN]
        fblk = np.concatenate(
            [fb, np.ones((128, NT, 1), dtype=np.float16)], axis=2
        ).reshape(128, NT * (C_IN + 1))
        fblk = np.ascontiguousarray(fblk)
        adjm_b = ((adj[b] - 1.0) * SHIFT).astype(np.float16)  # [T, T]
        for cc in range(CPB):
            r0 = cc * ROWS
            ks = kb[r0 : r0 + ROWS].reshape(ROWS // 2, 2, D)
            kpair = np.ascontiguousarray(
                np.concatenate([ks[:, 0, :].T, ks[:, 1, :].T], axis=0).astype(
                    np.float32
                )
            )  # [128, ROWS//2]
            in_maps.append(
                {
                    "qT2": qT2,
                    "kpair": kpair,
                    "a32": a32,
                    "feat16b": fblk,
                    "adjm": np.ascontiguousarray(adjm_b[r0 : r0 + ROWS]),
                }
            )
    return in_maps


def run(feat, adj, W1, W2, a, trace=False):
    nc = build_program()
    in_maps = make_in_maps(feat, adj, W1, W2, a)
    last_err = None
    for attempt in range(3):
        try:
            res = run_bass_kernel_spmd(
                nc, in_maps, core_ids=list(range(N_CORES)), trace=trace
            )
            outs = [np.asarray(res.results[c]["out"]) for c in range(N_CORES)]
            break
        except Exception as e:  # transient NRT device errors recover on retry
            last_err = e
            import time

            time.sleep(5)
    else:
        raise last_err
    full = np.concatenate(outs, axis=0).reshape(B, T, C_IN).astype(np.float32)
    return full, res


def kernel(feat, adj, W1, W2, a):
    full, _ = run(feat, adj, W1, W2, a)
    return full


# revision 12
# speedup vs baseline: 7.4870x; 7.4870x over previous
"""GATv2 attention layer (B=2, T=1024, C_IN=128, D=64) on 8 trn2 NeuronCores.

Sharding: flatten (B, T) destination rows -> 2048 rows, 256 per core.
q = feat@W1.T and k = feat@W2.T are computed on host (tiny O(T*C) work); the
device does all O(T^2) work. Per-core fp16 host layouts: qT2 = [q^T; q^T]
stacked [128(=2x64 d), 1024(=j)], kpair columns [k(2p); k(2p+1)], the score
weight A32s, feat in 128-row blocks with a ones column (final matmul rhs +
free row-sum), and adjm = (adj-1)*SHIFT (additive softmax mask).

Per-core algorithm (i = destination row, j = source node, d = head dim 64):
  scores[i, j] = sum_d a[d] * relu(q[j, d] + k[i, d])
For a PAIR of rows (2p, 2p+1), bias column kpair[:, p] = [k[2p]; k[2p+1]]:
  E2 = relu(qT2 + kpair[:, p])   one elementwise op, round-robined across
  VectorE (DVE tensor_scalar, 4x fp16 mode), ScalarE (Relu activation) and
  GpSimd (Q7 tensor_scalar) so all three produce E2 tiles concurrently.
Scores come from PE matmuls with lhsT = A32s slot q, a [128, 32] fp16 matrix
holding `a` in columns 2q (top d-half) and 2q+1 (bottom); 16 pairs accumulate
into one 32-row psum band; consecutive pairs hit different PSUM col-groups so
the matmuls overlap on the PE sub-arrays.
The adjacency mask is folded into the psum accumulation: each i-tile starts
with psum := I^T @ adjm (start=True), so masked scores sit at s - SHIFT and
softmax needs no separate multiply:
  att_unnorm = exp(s - BSUB)    (BSUB keeps exp in fp16 range; softmax is
  shift-invariant so the result is exact in fp32 terms)
attT via xbar DMA transpose (blocked 128x128), final: out[i, :] =
(attT.T @ [feat | 1]) with the ones column giving the row-sum, then scale by
its reciprocal.
"""
import sys

sys.path.insert(0, "/opt/trn_rl_repo")

from contextlib import ExitStack

import numpy as np

import concourse.bass as bass  # noqa: F401
import concourse.tile as tile
from concourse import bacc, masks, mybir
from concourse.bass_utils import run_bass_kernel_spmd

B, T, C_IN, D = 2, 1024, 128, 64
N_CORES = 8
ROWS = (B * T) // N_CORES  # 256 destination rows per core
CPB = N_CORES // B  # cores per batch
NT = T // 128  # token tiles
NIT = ROWS // 128  # i-tiles per core
NPAIR = 64  # row pairs per i-tile
NSLOT = 16  # pair slots per 32-row psum band

FP32 = mybir.dt.float32
FP16 = mybir.dt.float16
AX = mybir.AxisListType.X
OP = mybir.AluOpType
AF = mybir.ActivationFunctionType

SHIFT = 40.0  # additive mask magnitude: masked scores -> s - SHIFT
BSUB = 4.0  # global exp shift: att_unnorm = exp(s - BSUB), keeps fp16 range

# Per-i-tile engine assignment for the 64 E2 tiles: greedy balance by
# measured per-tile cost (DVE 396 ns, ScalarE 1150 ns) with initial queue
# offsets (ScalarE pays the ACT table load / exp of the previous i-tile).
# GpSimd's stock tensor_scalar measured ~15 us/tile on HW — unusable.
def _make_pattern(engines="VS", wv=396, ws=1150, wg=15000, pv=500, ps=3000, pg=1100):
    cost = {"V": wv, "S": ws, "G": wg}
    load = {"V": pv, "S": ps, "G": pg}
    pat = []
    for _ in range(NPAIR):
        e = min(engines, key=lambda x: load[x] + cost[x])
        load[e] += cost[e]
        pat.append(e)
    return "".join(pat)


PATTERN = _make_pattern()
PAIRS_PER_ALLOC = 4


def _emit(ctx, tc, nc, qT2_in, kpair_in, a32_in, feat16b_in, adjm_in, out):
    singles = ctx.enter_context(tc.tile_pool(name="singles", bufs=1))
    ident16 = singles.tile([128, 128], FP16)
    qT2 = singles.tile([128, T], FP16)
    kpair = singles.tile([128, ROWS // 2], FP32)
    A32s = singles.tile([128, NSLOT * 32], FP16)
    feat16 = singles.tile([128, NT * (C_IN + 1)], FP16)
    bsub = singles.tile([128, 1], FP32)
    nc.vector.memset(bsub[:], -BSUB)

    # prologue: parallel DMAs on the three DMA-capable queues (sync, scalar,
    # gpsimd); critical path is qT2+kpair
    nc.sync.dma_start(qT2[:], qT2_in[:, :])
    nc.sync.dma_start(kpair[:], kpair_in[:, :])
    nc.scalar.dma_start(A32s[:], a32_in[:, :])
    nc.scalar.dma_start(feat16[:], feat16b_in[:, :])
    masks.make_identity(nc, ident16[:])

    adjpool = ctx.enter_context(tc.tile_pool(name="adjp", bufs=2))
    e2pool = ctx.enter_context(tc.tile_pool(name="e2", bufs=4))
    pattpool = ctx.enter_context(tc.tile_pool(name="patt", bufs=2))
    attTpool = ctx.enter_context(tc.tile_pool(name="attT", bufs=2))
    outpool = ctx.enter_context(tc.tile_pool(name="outp", bufs=2))
    smallpool = ctx.enter_context(tc.tile_pool(name="small", bufs=2))
    ps_scores = ctx.enter_context(tc.tile_pool(name="ps_s", bufs=4, space="PSUM"))
    ps_out = ctx.enter_context(tc.tile_pool(name="ps_o", bufs=2, space="PSUM"))
    ps_tr = ctx.enter_context(tc.tile_pool(name="ps_tr", bufs=1, space="PSUM"))

    st = {}

    def load_adjm(it, eng):
        t = adjpool.tile([128, T], FP16, tag="adjm")
        eng.dma_start(t[:], adjm_in[it * 128 : (it + 1) * 128, :])
        st["adjm", it] = t

    def mask_mm(it):
        # psum := mask (0 / -SHIFT); scores accumulate on top
        s0 = ps_scores.tile([128, 512], FP32, tag="s")
        s1 = ps_scores.tile([128, 512], FP32, tag="s")
        adj_sb = st["adjm", it]
        nc.tensor.matmul(
            s0[:], ident16[:], adj_sb[:, 0:512], start=True, stop=False,
            skip_group_check=True,
        )
        nc.tensor.matmul(
            s1[:], ident16[:], adj_sb[:, 512:T], start=True, stop=False,
            skip_group_check=True,
        )
        st["s", it] = (s0, s1)

    def pairs(it, lo, hi):
        s0, s1 = st["s", it]
        for idx in range(lo, hi):
            q, g = divmod(idx, 4)
            p = NSLOT * g + q
            P = it * NPAIR + p
            sub = idx % PAIRS_PER_ALLOC
            if sub == 0:
                st["e2big"] = e2pool.tile(
                    [128, PAIRS_PER_ALLOC * T], FP16, tag="e2", name="e2big"
                )
            e2 = st["e2big"][:, sub * T : (sub + 1) * T]
            kcol = kpair[:, P : P + 1]
            eng = PATTERN[idx]
            if eng == "S":
                nc.scalar.activation(e2[:], qT2[:], AF.Relu, bias=kcol)
            elif eng == "G":
                nc.gpsimd.tensor_scalar(e2[:], qT2[:], kcol, 0.0, OP.add, OP.max)
            else:
                nc.vector.tensor_scalar(e2[:], qT2[:], kcol, 0.0, OP.add, OP.max)
            lhsT = A32s[:, 32 * q : 32 * q + 32]
            last = q == NSLOT - 1
            nc.tensor.matmul(
                s0[32 * g : 32 * g + 32, :], lhsT, e2[:, 0:512],
                start=False, stop=last,
                tile_position=(0, 32 * g), skip_group_check=True,
            )
            nc.tensor.matmul(
                s1[32 * g : 32 * g + 32, :], lhsT, e2[:, 512:T],
                start=False, stop=last,
                tile_position=(0, 32 * g), skip_group_check=True,
            )

    def tail_exp(it):
        s0, s1 = st["s", it]
        patt = pattpool.tile([128, T], FP16, tag="patt")
        nc.scalar.activation(patt[:, 0:512], s0[:], AF.Exp, bias=bsub[:])
        nc.scalar.activation(patt[:, 512:T], s1[:], AF.Exp, bias=bsub[:])
        st["patt", it] = patt

    def tail_out(it):
        patt = st["patt", it]
        attT = attTpool.tile([128, T], FP16, tag="attT")
        # per-block 128x128 transposes via the DMA xbar on the (idle) sync
        # queue: keeps both PE and DVE free for the concurrent pair loop
        for t in range(NT):
            nc.sync.dma_start_transpose(
                attT[:, t * 128 : (t + 1) * 128], patt[:, t * 128 : (t + 1) * 128]
            )
        W = C_IN + 1
        po = ps_out.tile([128, W], FP32, tag="o")
        for t in range(NT):
            nc.tensor.matmul(
                po[:], attT[:, t * 128 : (t + 1) * 128],
                feat16[:, t * W : (t + 1) * W],
                start=(t == 0), stop=(t == NT - 1),
            )
        inv = smallpool.tile([128, 1], FP32, tag="inv")
        nc.vector.reciprocal(inv[:], po[:, C_IN : C_IN + 1])
        out_sb = outpool.tile([128, C_IN], FP32, tag="out")
        nc.vector.tensor_scalar(out_sb[:], po[:, 0:C_IN], inv[:], None, OP.mult)
        nc.sync.dma_start(out[it * 128 : (it + 1) * 128, :], out_sb[:])

    def tail_last(it):
        # exposed end-of-kernel tail: chunk finely and use PE transposes +
        # DVE copies (all engines are otherwise idle here; latency wins)
        s0, s1 = st["s", it]
        patt = pattpool.tile([128, T], FP16, tag="patt")
        pst = ps_tr.tile([128, T], FP16, tag="tr")
        attT = attTpool.tile([128, T], FP16, tag="attT")
        W = C_IN + 1
        po = ps_out.tile([128, W], FP32, tag="o")
        for hh in range(4):
            lo = hh * 256
            src = (s0, s1)[hh // 2]
            slo = lo % 512
            nc.scalar.activation(
                patt[:, lo : lo + 256], src[:, slo : slo + 256], AF.Exp,
                bias=bsub[:],
            )
            for t in range(lo // 128, lo // 128 + 2):
                nc.tensor.transpose(
                    pst[:, t * 128 : (t + 1) * 128],
                    patt[:, t * 128 : (t + 1) * 128], ident16[:],
                )
            nc.vector.tensor_copy(attT[:, lo : lo + 256], pst[:, lo : lo + 256])
            for t in range(lo // 128, lo // 128 + 2):
                nc.tensor.matmul(
                    po[:], attT[:, t * 128 : (t + 1) * 128],
                    feat16[:, t * W : (t + 1) * W],
                    start=(t == 0), stop=(t == NT - 1),
                )
        inv = smallpool.tile([128, 1], FP32, tag="inv")
        nc.vector.reciprocal(inv[:], po[:, C_IN : C_IN + 1])
        out_sb = outpool.tile([128, C_IN], FP32, tag="out")
        nc.vector.tensor_scalar(out_sb[:], po[:, 0:C_IN], inv[:], None, OP.mult)
        nc.sync.dma_start(out[it * 128 : (it + 1) * 128, :], out_sb[:])

    load_adjm(0, nc.sync)
    mask_mm(0)
    load_adjm(1, nc.gpsimd)
    pairs(0, 0, NPAIR)
    mask_mm(1)
    pairs(1, 0, 16)
    tail_exp(0)
    pairs(1, 16, 32)
    tail_out(0)
    pairs(1, 32, NPAIR)
    tail_last(1)


_PROGRAM = None


def build_program():
    global _PROGRAM
    if _PROGRAM is not None:
        return _PROGRAM
    nc = bacc.Bacc("TRN2", target_bir_lowering=False, debug=False, num_devices=N_CORES)
    qT2_in = nc.dram_tensor("qT2", [128, T], FP16, kind="ExternalInput")
    kpair_in = nc.dram_tensor("kpair", [128, ROWS // 2], FP32, kind="ExternalInput")
    a32_in = nc.dram_tensor("a32", [128, NSLOT * 32], FP16, kind="ExternalInput")
    feat16b_in = nc.dram_tensor(
        "feat16b", [128, NT * (C_IN + 1)], FP16, kind="ExternalInput"
    )
    adjm_in = nc.dram_tensor("adjm", [ROWS, T], FP16, kind="ExternalInput")
    out = nc.dram_tensor("out", [ROWS, C_IN], FP32, kind="ExternalOutput")
    with tile.TileContext(nc) as tc:
        with ExitStack() as ctx:
            _emit(ctx, tc, nc, qT2_in, kpair_in, a32_in, feat16b_in, adjm_in, out)
    nc.compile()
    _PROGRAM = nc
    return nc


def make_a32(a):
    a32 = np.zeros((128, NSLOT * 32), dtype=np.float16)
    for q in range(NSLOT):
        a32[0:64, 32 * q + 2 * q] = a
        a32[64:128, 32 * q + 2 * q + 1] = a
    return a32


def make_in_maps(feat, adj, W1, W2, a):
    feat = np.ascontiguousarray(feat, dtype=np.float32)
    adj = np.ascontiguousarray(adj, dtype=np.float32)
    W1 = np.asarray(W1, dtype=np.float32)
    W2 = np.asarray(W2, dtype=np.float32)
    a32 = make_a32(np.asarray(a, dtype=np.float32))
    in_maps = []
    for b in range(B):
        qb = feat[b] @ W1.T  # [T, D]
        kb = feat[b] @ W2.T  # [T, D]
        qT = np.ascontiguousarray(qb.T.astype(np.float16))  # [D, T]
        qT2 = np.ascontiguousarray(np.concatenate([qT, qT], axis=0))  # [128, T]
        feat16 = feat[b].astype(np.float16)
        fb = feat16.reshape(NT, 128, C_IN).transpose(1, 0, 2)  # [128, NT, C_IN]
        fblk = np.concatenate(
            [fb, np.ones((128, NT, 1), dtype=np.float16)], axis=2
        ).reshape(128, NT * (C_IN + 1))
        fblk = np.ascontiguousarray(fblk)
        adjm_b = ((adj[b] - 1.0) * SHIFT).astype(np.float16)  # [T, T]
        for cc in range(CPB):
            r0 = cc * ROWS
            ks = kb[r0 : r0 + ROWS].reshape(ROWS // 2, 2, D)
            kpair = np.ascontiguousarray(
                np.concatenate([ks[:, 0, :].T, ks[:, 1, :].T], axis=0).astype(
                    np.float32
                )
            )  # [128, ROWS//2]
            in_maps.append(
                {
                    "qT2": qT2,
                    "kpair": kpair,
                    "a32": a32,
                    "feat16b": fblk,
                    "adjm": np.ascontiguousarray(adjm_b[r0 : r0 + ROWS]),
                }
            )
    return in_maps


def run(feat, adj, W1, W2, a, trace=False):
    nc = build_program()
    in_maps = make_in_maps(feat, adj, W1, W2, a)
    last_err = None
    for attempt in range(3):
        try:
            res = run_bass_kernel_spmd(
                nc, in_maps, core_ids=list(range(N_CORES)), trace=trace
            )
            outs = [np.asarray(res.results[c]["out"]) for c in range(N_CORES)]
            break
        except Exception as e:  # transient NRT device errors recover on retry
            last_err = e
            import time

            time.sleep(5)
    else:
        raise last_err
    full = np.concatenate(outs, axis=0).reshape(B, T, C_IN).astype(np.float32)
    return full, res


def kernel(feat, adj, W1, W2, a):
    full, _ = run(feat, adj, W1, W2, a)
    return full
